# revision 33
# baseline (speedup 1.0000x reference)
"""Trainium2 Bass kernel for BatchTaskAlignedAssigner (topk_masking).

Strategy: pure data parallelism — batch dim B=32 sharded as 4 images per
NeuronCore across 8 cores.  Inside each core everything is computed in a
"P-major" layout: priors tiled as 120 partitions x 70 chunks, with the
40 GT boxes on the free dimension (per-gt values broadcast across
partitions once per image, per-prior values broadcast along the free dim
via stride-0 access patterns).  The top-13 selection runs in "G-major"
layout ((image,gt) rows x 8400 priors) on a PE-transposed copy of the
metrics using the DVE max8/match_replace instructions, whose
first-occurrence tie semantics match jax.lax.top_k exactly.
"""

import numpy as np

import concourse.bass as bass
import concourse.mybir as mybir
import concourse.tile as tile
from concourse import bass_isa
from concourse.bass_utils import run_bass_kernel_spmd
import drain_patch
drain_patch.install()

F32 = mybir.dt.float32
I32 = mybir.dt.int32
U16 = mybir.dt.uint16
U8 = mybir.dt.uint8

B, P, G, C = 32, 8400, 40, 80
TOPK = 13
EPS = 1e-7
NCORE = 8
IPC = B // NCORE          # images per core = 4
PP = 120                  # partitions used for the prior tiling
NCH = P // PP             # 70 chunks
NB = 35                   # chunks per phase-A block
NBLK = NCH // NB          # 5 blocks
DEBUG = False
TRACE = False
RUN_KWARGS = None
LAST_RESULT = None
_DBG = {}
A = mybir.AluOpType
AF = mybir.ActivationFunctionType
AX = mybir.AxisListType


def _emit(nc: bass.Bass):
    pb = nc.dram_tensor("pb", [IPC, P, 4], F32, kind="ExternalInput")
    ps = nc.dram_tensor("ps", [IPC, P, C], F32, kind="ExternalInput")
    pri = nc.dram_tensor("pri", [P, 4], F32, kind="ExternalInput")
    gl = nc.dram_tensor("gl", [IPC, G], I32, kind="ExternalInput")
    gb = nc.dram_tensor("gb", [IPC, G, 4], F32, kind="ExternalInput")
    pad = nc.dram_tensor("pad", [IPC, G], F32, kind="ExternalInput")
    cst = nc.dram_tensor("cst", [128, 289], F32, kind="ExternalInput")
    gtsc = nc.dram_tensor("gtsc", [IPC, 14 * G], F32)
    rrsc = nc.dram_tensor("rrsc", [IPC, G], F32)

    olab = nc.dram_tensor("olab", [IPC, P], I32, kind="ExternalOutput")
    global _DBG
    _DBG = {}
    if DEBUG:
        _DBG["dmtg"] = nc.dram_tensor("dmtg", [IPC, G, P], F32, kind="ExternalOutput")
        _DBG["dov"] = nc.dram_tensor("dov", [IPC, P, G], F32, kind="ExternalOutput")
        _DBG["dbs"] = nc.dram_tensor("dbs", [IPC, P, G], F32, kind="ExternalOutput")
        _DBG["dig"] = nc.dram_tensor("dig", [IPC, P, G], F32, kind="ExternalOutput")
        _DBG["dal"] = nc.dram_tensor("dal", [IPC, P, G], F32, kind="ExternalOutput")
        _DBG["dpos"] = nc.dram_tensor("dpos", [IPC, P, G], F32, kind="ExternalOutput")
    obox = nc.dram_tensor("obox", [IPC, P, 4], F32, kind="ExternalOutput")
    osco = nc.dram_tensor("osco", [IPC, P, C], F32, kind="ExternalOutput")
    ofg = nc.dram_tensor("ofg", [IPC, P], U8, kind="ExternalOutput")

    with tile.TileContext(nc) as tc:
        _body(tc, pb, ps, pri, gl, gb, pad, cst, gtsc, rrsc,
              olab, obox, osco, ofg)
    drain_patch.fix_multiwait(nc)
    return nc


def _body(tc, pb, ps, pri, gl, gb, pad, cst, gtsc, rrsc, olab, obox, osco, ofg):
    nc = tc.nc
    from contextlib import ExitStack

    ctx = ExitStack()
    with ctx:
        singles = ctx.enter_context(tc.tile_pool(name="singles", bufs=1))
        perimg = ctx.enter_context(tc.tile_pool(name="perimg", bufs=1))
        scrp = ctx.enter_context(tc.tile_pool(name="scrp", bufs=1))
        small = ctx.enter_context(tc.tile_pool(name="small", bufs=2))
        temps = ctx.enter_context(tc.tile_pool(name="temps", bufs=1))
        outp = ctx.enter_context(tc.tile_pool(name="outp", bufs=2))
        psum = ctx.enter_context(tc.tile_pool(name="psum", bufs=2, space="PSUM"))
        psum1 = ctx.enter_context(tc.tile_pool(name="psum1", bufs=1, space="PSUM"))

        # ---------------- one-time constants (host-provided) ----------------
        # cst packs: identity [128,128] | clsf [128,80] | giof [128,40]
        #            | gdesc [128,40] | piota [128,1]
        cstt = singles.tile([128, 289], F32)
        nc.sync.dma_start(out=cstt, in_=cst[:, :])
        ident = cstt[:, 0:128]
        clsf = cstt[:, 128:208]
        giof = cstt[:, 208:248]
        gdesc = cstt[:, 248:288]
        piota = cstt[:, 288:289]

        # priors x/y in P-major: PXY[q, j, c] = priors[c*PP+q, j]
        pxy = singles.tile([PP, 2, NCH], F32)
        for j in range(2):
            nc.sync.dma_start(out=pxy[:, j, :],
                              in_=pri.rearrange("(c q) j -> q j c", q=PP)[:, j, :])

        mtg = singles.tile([G, P], F32)   # G-major metrics rows for one image

        for img in range(IPC):
            # ---------------- per-gt preparation ----------------
            gbt = small.tile([G, 4], F32, tag="gbt")
            nc.sync.dma_start(out=gbt, in_=gb[img])
            glt = small.tile([G, 1], I32, tag="glt")
            nc.sync.dma_start(out=glt, in_=gl[img, :, None])
            padt = small.tile([G, 1], F32, tag="padt")
            nc.sync.dma_start(out=padt, in_=pad[img, :, None])

            # derived per-gt columns  [G, 13]
            # 0 x1t 1 y1t 2 x2t 3 y2t 4 a2e 5 cxt2 6 cyt2 7 atan2
            # 8 igx1 9 igy1 10 igx2 11 igy2 12 lbl
            der = small.tile([G, 13], F32, tag="der")
            nc.vector.tensor_copy(der[:, 0:4], gbt)
            w2 = small.tile([G, 4], F32, tag="w2")  # w2, h2, h2e, rh2
            nc.vector.tensor_tensor(out=w2[:, 0:1], in0=gbt[:, 2:3], in1=gbt[:, 0:1], op=A.subtract)
            nc.vector.tensor_tensor(out=w2[:, 1:2], in0=gbt[:, 3:4], in1=gbt[:, 1:2], op=A.subtract)
            nc.vector.tensor_scalar_add(w2[:, 2:3], w2[:, 1:2], EPS)
            nc.vector.reciprocal(out=w2[:, 3:4], in_=w2[:, 2:3])
            t0 = small.tile([G, 2], F32, tag="t0")
            nc.vector.tensor_tensor(out=t0[:, 0:1], in0=w2[:, 0:1], in1=w2[:, 1:2], op=A.mult)
            nc.vector.tensor_scalar_add(der[:, 4:5], t0[:, 0:1], EPS)       # a2e
            nc.vector.tensor_tensor(out=der[:, 5:6], in0=gbt[:, 0:1], in1=gbt[:, 2:3], op=A.add)
            nc.vector.tensor_tensor(out=der[:, 6:7], in0=gbt[:, 1:2], in1=gbt[:, 3:4], op=A.add)
            nc.vector.tensor_tensor(out=t0[:, 1:2], in0=w2[:, 0:1], in1=w2[:, 3:4], op=A.mult)
            # full-range arctan for x>0: atan(x); x>1 -> pi/2 - atan(1/x)
            at2 = small.tile([G, 3], F32, tag="at2")   # rx, r, alt
            nc.vector.reciprocal(out=at2[:, 0:1], in_=t0[:, 1:2])
            nc.vector.tensor_tensor(out=at2[:, 1:2], in0=t0[:, 1:2], in1=at2[:, 0:1], op=A.min)
            nc.scalar.activation(out=der[:, 7:8], in_=at2[:, 1:2], func=AF.Arctan)
            nc.vector.tensor_scalar(out=at2[:, 2:3], in0=der[:, 7:8], scalar1=-1.0,
                                    scalar2=1.5707963267948966, op0=A.mult, op1=A.add)
            amsk = small.tile([G, 1], U8, tag="amsk")
            nc.vector.tensor_scalar(out=amsk, in0=t0[:, 1:2], scalar1=1.0, scalar2=None,
                                    op0=A.is_gt)
            nc.vector.copy_predicated(out=der[:, 7:8], mask=amsk, data=at2[:, 2:3])
            # pad masking: big = 1e30 where pad <= 0
            big = small.tile([G, 1], F32, tag="big")
            nc.vector.tensor_scalar(out=big, in0=padt, scalar1=0.0, scalar2=1e30,
                                    op0=A.is_le, op1=A.mult)
            nc.vector.tensor_tensor(out=der[:, 8:9], in0=gbt[:, 0:1], in1=big, op=A.add)
            nc.vector.tensor_tensor(out=der[:, 9:10], in0=gbt[:, 1:2], in1=big, op=A.add)
            nc.vector.tensor_tensor(out=der[:, 10:11], in0=gbt[:, 2:3], in1=big, op=A.subtract)
            nc.vector.tensor_tensor(out=der[:, 11:12], in0=gbt[:, 3:4], in1=big, op=A.subtract)
            nc.vector.tensor_copy(der[:, 12:13], glt)   # int -> float cast

            # transpose [G,13] -> [13,G], bounce through DRAM to broadcast
            # across partitions (DRAM sources may use partition-step 0)
            dtp = psum1.tile([13, G], F32, tag="dtp")
            nc.tensor.transpose(dtp, der, ident[0:G, 0:G])
            rows13 = small.tile([13, G], F32, tag="rows13")
            nc.scalar.copy(rows13, dtp)
            nc.sync.dma_start(out=gtsc[img, 0:13 * G], in_=rows13)
            gt = small.tile([PP, 13, G], F32, tag="gt")
            gt_src = bass.AP(tensor=gtsc.ap().tensor, offset=img * 14 * G,
                             ap=[[0, PP], [1, 13 * G]])
            nc.sync.dma_start(out=gt.rearrange("p a b -> p (a b)"), in_=gt_src)


            # ---------------- per-prior preparation ----------------
            pred = small.tile([PP, NCH, 4], F32, tag="pred")
            nc.sync.dma_start(out=pred, in_=pb[img].rearrange("(c q) j -> q c j", q=PP))
            # perp planes: 0 area1, 1 cxp2, 2 cyp2, 3 atan1
            perp = small.tile([PP, 4, NCH], F32, tag="perp")
            w1 = small.tile([PP, 2, NCH], F32, tag="w1")
            nc.vector.tensor_tensor(out=w1[:, 0, :], in0=pred[:, :, 2], in1=pred[:, :, 0], op=A.subtract)
            nc.vector.tensor_tensor(out=w1[:, 1, :], in0=pred[:, :, 3], in1=pred[:, :, 1], op=A.subtract)
            nc.vector.tensor_tensor(out=perp[:, 0, :], in0=w1[:, 0, :], in1=w1[:, 1, :], op=A.mult)
            nc.vector.tensor_tensor(out=perp[:, 1, :], in0=pred[:, :, 0], in1=pred[:, :, 2], op=A.add)
            nc.vector.tensor_tensor(out=perp[:, 2, :], in0=pred[:, :, 1], in1=pred[:, :, 3], op=A.add)
            rh1 = small.tile([PP, 1, NCH], F32, tag="rh1")
            nc.vector.tensor_scalar_add(rh1[:, 0, :], w1[:, 1, :], EPS)
            nc.vector.reciprocal(out=rh1[:, 0, :], in_=rh1[:, 0, :])
            nc.vector.tensor_tensor(out=rh1[:, 0, :], in0=w1[:, 0, :], in1=rh1[:, 0, :], op=A.mult)
            atp = small.tile([PP, 3, NCH], F32, tag="atp")   # rx, r, alt
            nc.vector.reciprocal(out=atp[:, 0, :], in_=rh1[:, 0, :])
            nc.vector.tensor_tensor(out=atp[:, 1, :], in0=rh1[:, 0, :], in1=atp[:, 0, :], op=A.min)
            nc.scalar.activation(out=perp[:, 3, :], in_=atp[:, 1, :], func=AF.Arctan)
            nc.vector.tensor_scalar(out=atp[:, 2, :], in0=perp[:, 3, :], scalar1=-1.0,
                                    scalar2=1.5707963267948966, op0=A.mult, op1=A.add)
            amskp = small.tile([PP, 1, NCH], U8, tag="amskp")
            nc.vector.tensor_scalar(out=amskp[:, 0, :], in0=rh1[:, 0, :], scalar1=1.0,
                                    scalar2=None, op0=A.is_gt)
            nc.vector.copy_predicated(out=perp[:, 3, :], mask=amskp[:, 0, :], data=atp[:, 2, :])

            # ---------------- scores load + per-gt class gather ----------------
            # bbox_scores[p, g] = scores[p, label_g] via PE: transpose the
            # score chunk then multiply with a one-hot class-selection matrix.
            scr = scrp.tile([PP, NCH, C], F32, tag="scr")
            nc.sync.dma_start(out=scr,
                              in_=ps[img].rearrange("(c q) k -> q c k", q=PP))
            oh80 = small.tile([C, G], F32, tag="oh80")
            nc.vector.tensor_tensor(out=oh80,
                                    in0=piota[0:C].to_broadcast([C, G]),
                                    in1=gt[0:C, 12, :], op=A.is_equal)
            bs = perimg.tile([PP, NCH, G], F32, tag="bsmark")
            for c4 in range(0, NCH, 4):
                cw = min(4, NCH - c4)
                tsc = psum1.tile([C, 4, PP], F32, tag="tsc")
                for j in range(cw):
                    nc.tensor.transpose(tsc[:, j, :], scr[:, c4 + j, :],
                                        ident[0:PP, 0:PP])
                sct = small.tile([C, 4, PP], F32, tag="sct")
                nc.scalar.copy(sct[:, 0:cw, :], tsc[:, 0:cw, :])
                bsp = psum1.tile([PP, 4, G], F32, tag="bsp")
                for j in range(cw):
                    nc.tensor.matmul(bsp[:, j, :], lhsT=sct[:, j, :], rhs=oh80)
                nc.scalar.copy(bs[:, c4:c4 + cw, :], bsp[:, 0:cw, :])

            # ---------------- phase A: dense cross compute ----------------
            ov = perimg.tile([PP, NCH, G], F32, tag="ov")
            al = perimg.tile([PP, NCH, G], F32, tag="al")
            ig = perimg.tile([PP, NCH, G], F32, tag="ig")

            def gtv(k, n=NB):
                return gt[:, k:k + 1, :].to_broadcast([PP, n, G])

            for blk in range(NBLK):
                c0 = blk * NB
                c1 = c0 + NB
                sh = [PP, NB, G]

                def ppv(j):  # pred coord j broadcast over g
                    return pred[:, c0:c1, j:j + 1].to_broadcast(sh)

                def pev(k):  # perp plane k broadcast over g
                    return perp[:, k, c0:c1][:, :, None].to_broadcast(sh)

                def pxv(j):  # prior coord broadcast over g
                    return pxy[:, j, c0:c1][:, :, None].to_broadcast(sh)

                t1 = temps.tile(sh, F32, tag="t1")
                t2 = temps.tile(sh, F32, tag="t2")
                t3 = temps.tile(sh, F32, tag="t3")
                t4 = temps.tile(sh, F32, tag="t4")
                t5 = temps.tile(sh, F32, tag="t5")
                t6 = temps.tile(sh, F32, tag="t6")
                ovs = ov[:, c0:c1, :]
                als = al[:, c0:c1, :]
                igs = ig[:, c0:c1, :]

                V, Gp, S = nc.vector, nc.vector, nc.scalar
                # intersection
                V.tensor_tensor(out=t1, in0=ppv(2), in1=gtv(0 + 2), op=A.min)   # min(x2p,x2t)
                Gp.tensor_tensor(out=t2, in0=ppv(0), in1=gtv(0), op=A.max)      # max(x1p,x1t)
                V.tensor_tensor(out=t3, in0=ppv(3), in1=gtv(3), op=A.min)       # min(y2p,y2t)
                Gp.tensor_tensor(out=t4, in0=ppv(1), in1=gtv(1), op=A.max)      # max(y1p,y1t)
                V.tensor_tensor(out=t1, in0=t1, in1=t2, op=A.subtract)          # ow
                Gp.tensor_tensor(out=t3, in0=t3, in1=t4, op=A.subtract)         # oh
                S.activation(out=t1, in_=t1, func=AF.Relu)                      # relu(ow)
                V.scalar_tensor_tensor(out=t1, in0=t3, scalar=0.0, in1=t1,
                                       op0=A.max, op1=A.mult)                   # ovl
                # union reciprocal
                Gp.tensor_tensor(out=t2, in0=pev(0), in1=gtv(4), op=A.add)      # w1h1 + (w2h2+eps)
                V.tensor_tensor(out=t2, in0=t2, in1=t1, op=A.subtract)          # union
                V.reciprocal(out=t2, in_=t2)
                V.tensor_tensor(out=t1, in0=t1, in1=t2, op=A.mult)              # iou
                # enclosing box diag^2
                Gp.tensor_tensor(out=t2, in0=ppv(2), in1=gtv(2), op=A.max)
                V.tensor_tensor(out=t4, in0=ppv(0), in1=gtv(0), op=A.min)
                Gp.tensor_tensor(out=t2, in0=t2, in1=t4, op=A.subtract)         # ew
                V.tensor_tensor(out=t4, in0=ppv(3), in1=gtv(3), op=A.max)
                Gp.tensor_tensor(out=t5, in0=ppv(1), in1=gtv(1), op=A.min)
                V.tensor_tensor(out=t4, in0=t4, in1=t5, op=A.subtract)          # eh
                S.square(t2, t2)
                S.square(t4, t4)
                Gp.tensor_tensor(out=t2, in0=t2, in1=t4, op=A.add)              # ew2+eh2
                V.tensor_scalar(out=t2, in0=t2, scalar1=4.0, scalar2=4.0 * EPS,
                                op0=A.mult, op1=A.add)
                V.reciprocal(out=t2, in_=t2)                                    # 0.25/(enc+eps)
                # center distance
                Gp.tensor_tensor(out=t4, in0=gtv(5), in1=pev(1), op=A.subtract)
                V.tensor_tensor(out=t5, in0=gtv(6), in1=pev(2), op=A.subtract)
                S.square(t4, t4)
                S.square(t5, t5)
                Gp.tensor_tensor(out=t4, in0=t4, in1=t5, op=A.add)              # 4*rho2
                V.tensor_tensor(out=t2, in0=t4, in1=t2, op=A.mult)              # rho2/enc
                # aspect-ratio penalty
                Gp.tensor_tensor(out=t4, in0=gtv(7), in1=pev(3), op=A.subtract)
                S.activation(out=t4, in_=t4, func=AF.Square, scale=0.6366197723675814)  # wh
                V.tensor_tensor(out=t5, in0=t4, in1=t1, op=A.subtract)          # wh - iou
                V.tensor_scalar_add(t5, t5, 1.0 + EPS)
                V.reciprocal(out=t5, in_=t5)
                S.square(t6, t4)                                                # wh^2
                Gp.tensor_tensor(out=t5, in0=t6, in1=t5, op=A.mult)             # alpha*wh
                V.tensor_tensor(out=t1, in0=t1, in1=t2, op=A.subtract)
                Gp.tensor_tensor(out=t1, in0=t1, in1=t5, op=A.subtract)         # ciou
                V.tensor_scalar(out=ovs, in0=t1, scalar1=0.0, scalar2=1.0,
                                op0=A.max, op1=A.min)                           # overlaps
                # align = score * ov^6
                S.square(t2, ovs)
                S.square(t4, t2)
                Gp.tensor_tensor(out=t4, in0=t4, in1=t2, op=A.mult)             # ov^6
                V.tensor_tensor(out=als, in0=bs[:, c0:c1, :], in1=t4, op=A.mult)
                # in-gts mask (exact delta form) -- pad folded into bounds
                Gp.tensor_tensor(out=t2, in0=pxv(0), in1=gtv(8), op=A.subtract)     # px-x1t
                V.scalar_tensor_tensor(out=t4, in0=gtv(10), scalar=0.0, in1=pxv(0),
                                       op0=A.bypass, op1=A.subtract)                # x2t-px
                Gp.tensor_tensor(out=t2, in0=t2, in1=t4, op=A.min)
                V.tensor_tensor(out=t4, in0=pxv(1), in1=gtv(9), op=A.subtract)      # py-y1t
                Gp.scalar_tensor_tensor(out=t5, in0=gtv(11), scalar=0.0, in1=pxv(1),
                                        op0=A.bypass, op1=A.subtract)               # y2t-py
                V.tensor_tensor(out=t4, in0=t4, in1=t5, op=A.min)
                Gp.tensor_tensor(out=t2, in0=t2, in1=t4, op=A.min)
                V.tensor_scalar(out=igs, in0=t2, scalar1=1e-9, scalar2=None,
                                op0=A.is_gt)
                # metrics
                mtt = temps.tile(sh, F32, tag="mtt")
                V.tensor_tensor(out=mtt, in0=als, in1=igs, op=A.mult)

                # forward transpose of metrics into G-major rows
                for cs in range(0, NB, 4):
                    cw = min(4, NB - cs)
                    tps = psum.tile([G, 4, PP], F32, tag="fwdt")
                    for j in range(cw):
                        nc.tensor.transpose(tps[:, j, :], mtt[:, cs + j, :],
                                            ident[0:PP, 0:PP])
                    cc = c0 + cs
                    nc.scalar.copy(
                        mtg[:, cc * PP:(cc + cw) * PP],
                        tps[:, 0:cw, :].rearrange("p a b -> p (a b)"))

            if DEBUG:
                nc.sync.dma_start(out=_DBG["dmtg"][img], in_=mtg)
                for nm, tl in [("dov", ov), ("dbs", bs), ("dig", ig), ("dal", al)]:
                    nc.sync.dma_start(
                        out=_DBG[nm][img].rearrange("(c q) g -> q c g", q=PP), in_=tl)
            # ---------------- top-13 (exact jax.lax.top_k semantics) ----------------
            m1 = small.tile([G, 8], F32, tag="m1")
            m2 = small.tile([G, 8], F32, tag="m2")
            nc.vector.max(out=m1, in_=mtg)
            nc.vector.match_replace(out=mtg, in_to_replace=m1, in_values=mtg,
                                    imm_value=-1.0)
            nc.vector.max(out=m2, in_=mtg)
            nc.vector.memset(m2[:, 5:8], -1.0)
            nc.vector.match_replace(out=mtg, in_to_replace=m2, in_values=mtg,
                                    imm_value=-1.0)

            # ---------------- transpose marks back to P-major ----------------
            mark = perimg.tile([PP, NCH, G], F32, tag="bsmark")
            for cs in range(0, NCH, 5):
                tps = psum.tile([PP, 5, G], F32, tag="bwdt")
                for j in range(5):
                    c = cs + j
                    nc.tensor.transpose(tps[:, j, :], mtg[:, c * PP:(c + 1) * PP],
                                        ident[0:G, 0:G])
                nc.scalar.copy(mark[:, cs:cs + 5, :], tps)

            # ---------------- assignment resolution ----------------
            V, Gp = nc.vector, nc.vector
            pos = mark  # in-place: pos = (mark == -1) * ig
            V.scalar_tensor_tensor(out=pos, in0=mark, scalar=-1.0, in1=ig,
                                   op0=A.is_equal, op1=A.mult)
            fgi = small.tile([PP, NCH], F32, tag="fgi")
            V.tensor_reduce(out=fgi, in_=pos, axis=AX.X, op=A.add)
            multi = small.tile([PP, NCH], F32, tag="multi")
            V.tensor_scalar(out=multi, in0=fgi, scalar1=1.0, scalar2=None, op0=A.is_gt)

            cmx = small.tile([PP, NCH], F32, tag="cmx")
            V.tensor_reduce(out=cmx, in_=ov, axis=AX.X, op=A.max)
            eqm = scr[0:PP, :, 0:G]  # scr is dead after the BS gather
            Gp.tensor_tensor(out=eqm, in0=ov,
                             in1=cmx[:, :, None].to_broadcast([PP, NCH, G]),
                             op=A.is_equal)
            Gp.tensor_tensor(out=eqm, in0=eqm,
                             in1=gdesc[0:PP, None, :].to_broadcast([PP, NCH, G]),
                             op=A.mult)
            bmx = small.tile([PP, NCH], F32, tag="bmx")
            V.tensor_reduce(out=bmx, in_=eqm, axis=AX.X, op=A.max)
            bestg = small.tile([PP, NCH], F32, tag="bestg")
            V.tensor_scalar(out=bestg, in0=bmx, scalar1=-1.0, scalar2=float(G),
                            op0=A.mult, op1=A.add)
            ismax = eqm
            Gp.tensor_tensor(out=ismax, in0=giof[0:PP, None, :].to_broadcast([PP, NCH, G]),
                             in1=bestg[:, :, None].to_broadcast([PP, NCH, G]),
                             op=A.is_equal)
            notm = small.tile([PP, NCH], F32, tag="notm")
            V.tensor_scalar(out=notm, in0=multi, scalar1=-1.0, scalar2=1.0,
                            op0=A.mult, op1=A.add)
            Gp.tensor_tensor(out=ismax, in0=ismax,
                             in1=multi[:, :, None].to_broadcast([PP, NCH, G]), op=A.mult)
            V.tensor_tensor(out=pos, in0=pos,
                            in1=notm[:, :, None].to_broadcast([PP, NCH, G]), op=A.mult)
            V.tensor_tensor(out=pos, in0=pos, in1=ismax, op=A.add)
            fgf = small.tile([PP, NCH], F32, tag="fgf")
            V.tensor_reduce(out=fgf, in_=pos, axis=AX.X, op=A.add)
            fgb = small.tile([PP, NCH], U8, tag="fgb")
            V.tensor_scalar(out=fgb, in0=fgf, scalar1=0.0, scalar2=None, op0=A.is_gt)
            if DEBUG:
                nc.sync.dma_start(
                    out=_DBG["dpos"][img].rearrange("(c q) g -> q c g", q=PP),
                    in_=pos)



            # gather gt box + label via one-hot contraction (pos has <=1
            # nonzero per prior); background keeps gt[0] like the reference.
            gsc = ig  # ig is dead after pos was built
            bxl = small.tile([PP, NCH, 5], F32, tag="bxl")
            planes = [0, 1, 2, 3, 12]
            for j, pk in enumerate(planes):
                eng = V if j % 2 == 0 else Gp
                eng.tensor_tensor(out=gsc, in0=pos,
                                  in1=gt[:, pk:pk + 1, :].to_broadcast([PP, NCH, G]),
                                  op=A.mult)
                sel = bxl[:, :, j]
                V.tensor_reduce(out=sel, in_=gsc, axis=AX.X, op=A.add)
                fill = small.tile([PP, NCH], F32, tag="fill")
                nc.scalar.copy(fill, gt[:, pk, 0:1].to_broadcast([PP, NCH]))
                V.copy_predicated(out=fill, mask=fgb, data=sel)
                V.tensor_copy(sel, fill)

            # ---------------- normalizer ----------------
            am = al
            Gp.tensor_tensor(out=am, in0=al, in1=pos, op=A.mult)
            om = ov
            Gp.tensor_tensor(out=om, in0=ov, in1=pos, op=A.mult)
            pvv = small.tile([PP, 2 * G], F32, tag="pvv")
            V.tensor_reduce(out=pvv[:, 0:G], in_=am.rearrange("p c g -> p g c"),
                            axis=AX.X, op=A.max)
            V.tensor_reduce(out=pvv[:, G:2 * G], in_=om.rearrange("p c g -> p g c"),
                            axis=AX.X, op=A.max)
            tpv = psum1.tile([2 * G, PP], F32, tag="tpv")
            nc.tensor.transpose(tpv, pvv, ident[0:PP, 0:PP])
            spv = small.tile([2 * G, PP], F32, tag="spv")
            nc.scalar.copy(spv, tpv)
            col = small.tile([2 * G, 1], F32, tag="col")
            V.tensor_reduce(out=col, in_=spv, axis=AX.X, op=A.max)
            tcol = psum1.tile([1, 2 * G], F32, tag="dtp")
            nc.tensor.transpose(tcol, col, ident[0:2 * G, 0:2 * G])
            rowpa = small.tile([1, 2 * G], F32, tag="rowpa")
            nc.scalar.copy(rowpa, tcol)
            rr = small.tile([1, G], F32, tag="rr")
            V.tensor_scalar_add(rr, rowpa[:, 0:G], EPS)
            V.reciprocal(out=rr, in_=rr)
            V.tensor_tensor(out=rr, in0=rowpa[:, G:2 * G], in1=rr, op=A.mult)
            rbc = small.tile([PP, G], F32, tag="rbc")
            nc.sync.dma_start(out=rrsc[img, :], in_=rr)
            rbc_src = bass.AP(tensor=rrsc.ap().tensor, offset=img * G,
                              ap=[[0, PP], [1, G]])
            nc.sync.dma_start(out=rbc, in_=rbc_src)
            Gp.tensor_tensor(out=am, in0=am,
                             in1=rbc[:, None, :].to_broadcast([PP, NCH, G]), op=A.mult)
            norm = small.tile([PP, NCH], F32, tag="norm")
            V.tensor_reduce(out=norm, in_=am, axis=AX.X, op=A.max)

            # ---------------- outputs ----------------
            labi = outp.tile([PP, NCH], I32, tag="labi")
            nc.vector.tensor_copy(labi, bxl[:, :, 4])
            nc.sync.dma_start(out=olab[img].rearrange("(c q) -> q c", q=PP), in_=labi)
            nc.sync.dma_start(out=obox[img].rearrange("(c q) j -> q c j", q=PP),
                              in_=bxl[:, :, 0:4])
            nc.sync.dma_start(out=ofg[img].rearrange("(c q) -> q c", q=PP), in_=fgb)

            for blk in range(NBLK):
                c0 = blk * NB
                c1 = c0 + NB
                eng = nc.vector
                sco = outp.tile([PP, NB, C], F32, tag="sco")
                eng.tensor_tensor(out=sco,
                                  in0=clsf[0:PP, None, :].to_broadcast([PP, NB, C]),
                                  in1=bxl[:, c0:c1, 4][:, :, None].to_broadcast([PP, NB, C]),
                                  op=A.is_equal)
                eng2 = nc.vector
                eng2.tensor_tensor(out=sco, in0=sco,
                                   in1=norm[:, c0:c1, None].to_broadcast([PP, NB, C]),
                                   op=A.mult)
                nc.sync.dma_start(
                    out=osco[img].rearrange("(c q) k -> q c k", q=PP)[:, c0:c1, :],
                    in_=sco)


_NC_CACHE = None


def _get_nc():
    global _NC_CACHE
    if _NC_CACHE is None:
        nc = bass.Bass()
        _NC_CACHE = _emit(nc)
    return _NC_CACHE


def _make_cst():
    cst = np.zeros((128, 289), np.float32)
    cst[:, 0:128] = np.eye(128, dtype=np.float32)
    cst[:, 128:208] = np.arange(C, dtype=np.float32)[None, :]
    cst[:, 208:248] = np.arange(G, dtype=np.float32)[None, :]
    cst[:, 248:288] = float(G) - np.arange(G, dtype=np.float32)[None, :]
    cst[:, 288] = np.arange(128, dtype=np.float32)
    return cst


def kernel(pred_bboxes, pred_scores, priors, gt_labels, gt_bboxes, pad_bbox_flag):
    lab_dt = gt_labels.dtype
    nc = _get_nc()
    in_maps = []
    for k in range(NCORE):
        s = slice(k * IPC, (k + 1) * IPC)
        in_maps.append({
            "pb": np.ascontiguousarray(pred_bboxes[s], np.float32),
            "ps": np.ascontiguousarray(pred_scores[s], np.float32),
            "pri": np.ascontiguousarray(priors, np.float32),
            "gl": np.ascontiguousarray(gt_labels[s, :, 0].astype(np.int32)),
            "gb": np.ascontiguousarray(gt_bboxes[s], np.float32),
            "pad": np.ascontiguousarray(pad_bbox_flag[s, :, 0], np.float32),
            "cst": _make_cst(),
        })
    res = run_bass_kernel_spmd(nc, in_maps, core_ids=list(range(NCORE)),
                               trace=TRACE, **(RUN_KWARGS or {}))
    global LAST_RESULT
    LAST_RESULT = res
    outs = res.results
    labels = np.concatenate([o["olab"] for o in outs], 0)
    bboxes = np.concatenate([o["obox"] for o in outs], 0)
    scores = np.concatenate([o["osco"] for o in outs], 0)
    fg = np.concatenate([o["ofg"] for o in outs], 0).astype(bool)
    if lab_dt != labels.dtype and np.issubdtype(lab_dt, np.integer):
        labels = labels.astype(np.int32)
    return labels, bboxes, scores, fg


# revision 34
# speedup vs baseline: 58.3618x; 58.3618x over previous
"""Trainium2 Bass kernel for BatchTaskAlignedAssigner (topk_masking).

Strategy: pure data parallelism — batch dim B=32 sharded as 4 images per
NeuronCore across 8 cores.  Inside each core everything is computed in a
"P-major" layout: priors tiled as 120 partitions x 70 chunks, with the
40 GT boxes on the free dimension (per-gt values broadcast across
partitions once per image, per-prior values broadcast along the free dim
via stride-0 access patterns).  The top-13 selection runs in "G-major"
layout ((image,gt) rows x 8400 priors) on a PE-transposed copy of the
metrics using the DVE max8/match_replace instructions, whose
first-occurrence tie semantics match jax.lax.top_k exactly.
"""

import numpy as np

import concourse.bass as bass
import concourse.mybir as mybir
import concourse.tile as tile
from concourse import bass_isa
from concourse.bass_utils import run_bass_kernel_spmd
# ---------------------------------------------------------------------------
# Compile workarounds for this walrus build: it encodes at most ONE sync wait
# per instruction. (a) Split the Tile kernel-tail drain into one drain per
# wait; (b) post-pass moving extra waits from any instruction onto inserted
# same-engine wait instructions.
# ---------------------------------------------------------------------------
from bass_rust import ScopedClock as _ScopedClock


def _drain_and_barrier_split(self, tick_clock, wait_clock):
    nc = self.nc
    d0 = nc.sync.drain()
    wait_clock.add_sem_waits(
        d0.ins, _ScopedClock({None: tick_clock.global_clock})
    )
    si0 = d0.ins.sync_info
    waits = list(si0.on_wait) if si0 is not None and si0.on_wait else []
    if len(waits) > 1:
        si0.on_wait = [waits[0]]
        for w in waits[1:]:
            di = nc.sync.drain()
            wait_clock.add_sem_waits(
                di.ins, _ScopedClock({None: tick_clock.global_clock})
            )
            di.ins.sync_info.on_wait = [w]
    nc.all_engine_barrier()
    assert self.sems is not None
    popped = nc._tile_sem_poison_stack.pop()
    assert popped is self._sem_poison
    nc.clear_and_free_semaphores(list(self.sems.allocated().values()))
    nc.all_engine_barrier()


tile.TileContext._drain_and_barrier = _drain_and_barrier_split


def _fix_multiwait(nc):
    blocks = list(nc.m.functions[0].blocks)
    snaps = [list(bb.instructions) for bb in blocks]
    for bb, snap in zip(blocks, snaps):
        out = []
        for inst in snap:
            si = inst.sync_info
            ws = list(si.on_wait) if si is not None and si.on_wait else []
            if len(ws) > 1:
                eng = nc.engines[inst.engine]
                for w in ws[:-1]:
                    assert w.wait_reg is None, f"register wait on {inst.name}"
                    sem = bass.SemaphoreHandle(w.ant_name, w.id)
                    nop = eng.wait_ge(sem, w.wait_value)
                    out.append(nop.ins)
                si.on_wait = [ws[-1]]
            out.append(inst)
        bb.instructions = out

F32 = mybir.dt.float32
I32 = mybir.dt.int32
U16 = mybir.dt.uint16
U8 = mybir.dt.uint8

B, P, G, C = 32, 8400, 40, 80
TOPK = 13
EPS = 1e-7
NCORE = 8
IPC = B // NCORE          # images per core = 4
PP = 120                  # partitions used for the prior tiling
NCH = P // PP             # 70 chunks
NB = 35                   # chunks per phase-A block
NBLK = NCH // NB          # 5 blocks
DEBUG = False
TRACE = False
RUN_KWARGS = None
LAST_RESULT = None
_DBG = {}
A = mybir.AluOpType
AF = mybir.ActivationFunctionType
AX = mybir.AxisListType


def _emit(nc: bass.Bass):
    pb = nc.dram_tensor("pb", [IPC, P, 4], F32, kind="ExternalInput")
    ps = nc.dram_tensor("ps", [IPC, P, C], F32, kind="ExternalInput")
    pri = nc.dram_tensor("pri", [P, 4], F32, kind="ExternalInput")
    gl = nc.dram_tensor("gl", [IPC, G], I32, kind="ExternalInput")
    gb = nc.dram_tensor("gb", [IPC, G, 4], F32, kind="ExternalInput")
    pad = nc.dram_tensor("pad", [IPC, G], F32, kind="ExternalInput")
    cst = nc.dram_tensor("cst", [128, 289], F32, kind="ExternalInput")
    gtsc = nc.dram_tensor("gtsc", [IPC, 14 * G], F32)
    rrsc = nc.dram_tensor("rrsc", [IPC, G], F32)

    olab = nc.dram_tensor("olab", [IPC, P], I32, kind="ExternalOutput")
    global _DBG
    _DBG = {}
    if DEBUG:
        _DBG["dmtg"] = nc.dram_tensor("dmtg", [IPC, G, P], F32, kind="ExternalOutput")
        _DBG["dov"] = nc.dram_tensor("dov", [IPC, P, G], F32, kind="ExternalOutput")
        _DBG["dbs"] = nc.dram_tensor("dbs", [IPC, P, G], F32, kind="ExternalOutput")
        _DBG["dig"] = nc.dram_tensor("dig", [IPC, P, G], F32, kind="ExternalOutput")
        _DBG["dal"] = nc.dram_tensor("dal", [IPC, P, G], F32, kind="ExternalOutput")
        _DBG["dpos"] = nc.dram_tensor("dpos", [IPC, P, G], F32, kind="ExternalOutput")
    obox = nc.dram_tensor("obox", [IPC, P, 4], F32, kind="ExternalOutput")
    osco = nc.dram_tensor("osco", [IPC, P, C], F32, kind="ExternalOutput")
    ofg = nc.dram_tensor("ofg", [IPC, P], U8, kind="ExternalOutput")

    with tile.TileContext(nc) as tc:
        _body(tc, pb, ps, pri, gl, gb, pad, cst, gtsc, rrsc,
              olab, obox, osco, ofg)
    _fix_multiwait(nc)
    return nc


def _body(tc, pb, ps, pri, gl, gb, pad, cst, gtsc, rrsc, olab, obox, osco, ofg):
    nc = tc.nc
    from contextlib import ExitStack

    ctx = ExitStack()
    with ctx:
        singles = ctx.enter_context(tc.tile_pool(name="singles", bufs=1))
        perimg = ctx.enter_context(tc.tile_pool(name="perimg", bufs=1))
        scrp = ctx.enter_context(tc.tile_pool(name="scrp", bufs=1))
        small = ctx.enter_context(tc.tile_pool(name="small", bufs=2))
        temps = ctx.enter_context(tc.tile_pool(name="temps", bufs=1))
        outp = ctx.enter_context(tc.tile_pool(name="outp", bufs=2))
        psum = ctx.enter_context(tc.tile_pool(name="psum", bufs=2, space="PSUM"))
        psum1 = ctx.enter_context(tc.tile_pool(name="psum1", bufs=1, space="PSUM"))

        # ---------------- one-time constants (host-provided) ----------------
        # cst packs: identity [128,128] | clsf [128,80] | giof [128,40]
        #            | gdesc [128,40] | piota [128,1]
        cstt = singles.tile([128, 289], F32)
        nc.sync.dma_start(out=cstt, in_=cst[:, :])
        ident = cstt[:, 0:128]
        clsf = cstt[:, 128:208]
        giof = cstt[:, 208:248]
        gdesc = cstt[:, 248:288]
        piota = cstt[:, 288:289]

        # priors x/y in P-major: PXY[q, j, c] = priors[c*PP+q, j]
        pxy = singles.tile([PP, 2, NCH], F32)
        for j in range(2):
            nc.sync.dma_start(out=pxy[:, j, :],
                              in_=pri.rearrange("(c q) j -> q j c", q=PP)[:, j, :])

        mtg = singles.tile([G, P], F32)   # G-major metrics rows for one image

        for img in range(IPC):
            # ---------------- per-gt preparation ----------------
            gbt = small.tile([G, 4], F32, tag="gbt")
            nc.sync.dma_start(out=gbt, in_=gb[img])
            glt = small.tile([G, 1], I32, tag="glt")
            nc.sync.dma_start(out=glt, in_=gl[img, :, None])
            padt = small.tile([G, 1], F32, tag="padt")
            nc.sync.dma_start(out=padt, in_=pad[img, :, None])

            # derived per-gt columns  [G, 13]
            # 0 x1t 1 y1t 2 x2t 3 y2t 4 a2e 5 cxt2 6 cyt2 7 atan2
            # 8 igx1 9 igy1 10 igx2 11 igy2 12 lbl
            der = small.tile([G, 13], F32, tag="der")
            nc.vector.tensor_copy(der[:, 0:4], gbt)
            w2 = small.tile([G, 4], F32, tag="w2")  # w2, h2, h2e, rh2
            nc.vector.tensor_tensor(out=w2[:, 0:1], in0=gbt[:, 2:3], in1=gbt[:, 0:1], op=A.subtract)
            nc.vector.tensor_tensor(out=w2[:, 1:2], in0=gbt[:, 3:4], in1=gbt[:, 1:2], op=A.subtract)
            nc.vector.tensor_scalar_add(w2[:, 2:3], w2[:, 1:2], EPS)
            nc.vector.reciprocal(out=w2[:, 3:4], in_=w2[:, 2:3])
            t0 = small.tile([G, 2], F32, tag="t0")
            nc.vector.tensor_tensor(out=t0[:, 0:1], in0=w2[:, 0:1], in1=w2[:, 1:2], op=A.mult)
            nc.vector.tensor_scalar_add(der[:, 4:5], t0[:, 0:1], EPS)       # a2e
            nc.vector.tensor_tensor(out=der[:, 5:6], in0=gbt[:, 0:1], in1=gbt[:, 2:3], op=A.add)
            nc.vector.tensor_tensor(out=der[:, 6:7], in0=gbt[:, 1:2], in1=gbt[:, 3:4], op=A.add)
            nc.vector.tensor_tensor(out=t0[:, 1:2], in0=w2[:, 0:1], in1=w2[:, 3:4], op=A.mult)
            # full-range arctan for x>0: atan(x); x>1 -> pi/2 - atan(1/x)
            at2 = small.tile([G, 3], F32, tag="at2")   # rx, r, alt
            nc.vector.reciprocal(out=at2[:, 0:1], in_=t0[:, 1:2])
            nc.vector.tensor_tensor(out=at2[:, 1:2], in0=t0[:, 1:2], in1=at2[:, 0:1], op=A.min)
            nc.scalar.activation(out=der[:, 7:8], in_=at2[:, 1:2], func=AF.Arctan)
            nc.vector.tensor_scalar(out=at2[:, 2:3], in0=der[:, 7:8], scalar1=-1.0,
                                    scalar2=1.5707963267948966, op0=A.mult, op1=A.add)
            amsk = small.tile([G, 1], U8, tag="amsk")
            nc.vector.tensor_scalar(out=amsk, in0=t0[:, 1:2], scalar1=1.0, scalar2=None,
                                    op0=A.is_gt)
            nc.vector.copy_predicated(out=der[:, 7:8], mask=amsk, data=at2[:, 2:3])
            # pad masking: big = 1e30 where pad <= 0
            big = small.tile([G, 1], F32, tag="big")
            nc.vector.tensor_scalar(out=big, in0=padt, scalar1=0.0, scalar2=1e30,
                                    op0=A.is_le, op1=A.mult)
            nc.vector.tensor_tensor(out=der[:, 8:9], in0=gbt[:, 0:1], in1=big, op=A.add)
            nc.vector.tensor_tensor(out=der[:, 9:10], in0=gbt[:, 1:2], in1=big, op=A.add)
            nc.vector.tensor_tensor(out=der[:, 10:11], in0=gbt[:, 2:3], in1=big, op=A.subtract)
            nc.vector.tensor_tensor(out=der[:, 11:12], in0=gbt[:, 3:4], in1=big, op=A.subtract)
            nc.vector.tensor_copy(der[:, 12:13], glt)   # int -> float cast

            # transpose [G,13] -> [13,G], bounce through DRAM to broadcast
            # across partitions (DRAM sources may use partition-step 0)
            dtp = psum1.tile([13, G], F32, tag="dtp")
            nc.tensor.transpose(dtp, der, ident[0:G, 0:G])
            rows13 = small.tile([13, G], F32, tag="rows13")
            nc.scalar.copy(rows13, dtp)
            nc.sync.dma_start(out=gtsc[img, 0:13 * G], in_=rows13)
            gt = small.tile([PP, 13, G], F32, tag="gt")
            gt_src = bass.AP(tensor=gtsc.ap().tensor, offset=img * 14 * G,
                             ap=[[0, PP], [1, 13 * G]])
            nc.sync.dma_start(out=gt.rearrange("p a b -> p (a b)"), in_=gt_src)


            # ---------------- per-prior preparation ----------------
            pred = small.tile([PP, NCH, 4], F32, tag="pred")
            nc.sync.dma_start(out=pred, in_=pb[img].rearrange("(c q) j -> q c j", q=PP))
            # perp planes: 0 area1, 1 cxp2, 2 cyp2, 3 atan1
            perp = small.tile([PP, 4, NCH], F32, tag="perp")
            w1 = small.tile([PP, 2, NCH], F32, tag="w1")
            nc.vector.tensor_tensor(out=w1[:, 0, :], in0=pred[:, :, 2], in1=pred[:, :, 0], op=A.subtract)
            nc.vector.tensor_tensor(out=w1[:, 1, :], in0=pred[:, :, 3], in1=pred[:, :, 1], op=A.subtract)
            nc.vector.tensor_tensor(out=perp[:, 0, :], in0=w1[:, 0, :], in1=w1[:, 1, :], op=A.mult)
            nc.vector.tensor_tensor(out=perp[:, 1, :], in0=pred[:, :, 0], in1=pred[:, :, 2], op=A.add)
            nc.vector.tensor_tensor(out=perp[:, 2, :], in0=pred[:, :, 1], in1=pred[:, :, 3], op=A.add)
            rh1 = small.tile([PP, 1, NCH], F32, tag="rh1")
            nc.vector.tensor_scalar_add(rh1[:, 0, :], w1[:, 1, :], EPS)
            nc.vector.reciprocal(out=rh1[:, 0, :], in_=rh1[:, 0, :])
            nc.vector.tensor_tensor(out=rh1[:, 0, :], in0=w1[:, 0, :], in1=rh1[:, 0, :], op=A.mult)
            atp = small.tile([PP, 3, NCH], F32, tag="atp")   # rx, r, alt
            nc.vector.reciprocal(out=atp[:, 0, :], in_=rh1[:, 0, :])
            nc.vector.tensor_tensor(out=atp[:, 1, :], in0=rh1[:, 0, :], in1=atp[:, 0, :], op=A.min)
            nc.scalar.activation(out=perp[:, 3, :], in_=atp[:, 1, :], func=AF.Arctan)
            nc.vector.tensor_scalar(out=atp[:, 2, :], in0=perp[:, 3, :], scalar1=-1.0,
                                    scalar2=1.5707963267948966, op0=A.mult, op1=A.add)
            amskp = small.tile([PP, 1, NCH], U8, tag="amskp")
            nc.vector.tensor_scalar(out=amskp[:, 0, :], in0=rh1[:, 0, :], scalar1=1.0,
                                    scalar2=None, op0=A.is_gt)
            nc.vector.copy_predicated(out=perp[:, 3, :], mask=amskp[:, 0, :], data=atp[:, 2, :])

            # ---------------- scores load + per-gt class gather ----------------
            # bbox_scores[p, g] = scores[p, label_g] via PE: transpose the
            # score chunk then multiply with a one-hot class-selection matrix.
            scr = scrp.tile([PP, NCH, C], F32, tag="scr")
            nc.sync.dma_start(out=scr,
                              in_=ps[img].rearrange("(c q) k -> q c k", q=PP))
            oh80 = small.tile([C, G], F32, tag="oh80")
            nc.vector.tensor_tensor(out=oh80,
                                    in0=piota[0:C].to_broadcast([C, G]),
                                    in1=gt[0:C, 12, :], op=A.is_equal)
            bs = perimg.tile([PP, NCH, G], F32, tag="bsmark")
            for c4 in range(0, NCH, 4):
                cw = min(4, NCH - c4)
                tsc = psum1.tile([C, 4, PP], F32, tag="tsc")
                for j in range(cw):
                    nc.tensor.transpose(tsc[:, j, :], scr[:, c4 + j, :],
                                        ident[0:PP, 0:PP])
                sct = small.tile([C, 4, PP], F32, tag="sct")
                nc.scalar.copy(sct[:, 0:cw, :], tsc[:, 0:cw, :])
                bsp = psum1.tile([PP, 4, G], F32, tag="bsp")
                for j in range(cw):
                    nc.tensor.matmul(bsp[:, j, :], lhsT=sct[:, j, :], rhs=oh80)
                nc.scalar.copy(bs[:, c4:c4 + cw, :], bsp[:, 0:cw, :])

            # ---------------- phase A: dense cross compute ----------------
            ov = perimg.tile([PP, NCH, G], F32, tag="ov")
            al = perimg.tile([PP, NCH, G], F32, tag="al")
            ig = perimg.tile([PP, NCH, G], F32, tag="ig")

            def gtv(k, n=NB):
                return gt[:, k:k + 1, :].to_broadcast([PP, n, G])

            for blk in range(NBLK):
                c0 = blk * NB
                c1 = c0 + NB
                sh = [PP, NB, G]

                def ppv(j):  # pred coord j broadcast over g
                    return pred[:, c0:c1, j:j + 1].to_broadcast(sh)

                def pev(k):  # perp plane k broadcast over g
                    return perp[:, k, c0:c1][:, :, None].to_broadcast(sh)

                def pxv(j):  # prior coord broadcast over g
                    return pxy[:, j, c0:c1][:, :, None].to_broadcast(sh)

                t1 = temps.tile(sh, F32, tag="t1")
                t2 = temps.tile(sh, F32, tag="t2")
                t3 = temps.tile(sh, F32, tag="t3")
                t4 = temps.tile(sh, F32, tag="t4")
                t5 = temps.tile(sh, F32, tag="t5")
                t6 = temps.tile(sh, F32, tag="t6")
                ovs = ov[:, c0:c1, :]
                als = al[:, c0:c1, :]
                igs = ig[:, c0:c1, :]

                V, Gp, S = nc.vector, nc.vector, nc.scalar
                # intersection
                V.tensor_tensor(out=t1, in0=ppv(2), in1=gtv(0 + 2), op=A.min)   # min(x2p,x2t)
                Gp.tensor_tensor(out=t2, in0=ppv(0), in1=gtv(0), op=A.max)      # max(x1p,x1t)
                V.tensor_tensor(out=t3, in0=ppv(3), in1=gtv(3), op=A.min)       # min(y2p,y2t)
                Gp.tensor_tensor(out=t4, in0=ppv(1), in1=gtv(1), op=A.max)      # max(y1p,y1t)
                V.tensor_tensor(out=t1, in0=t1, in1=t2, op=A.subtract)          # ow
                Gp.tensor_tensor(out=t3, in0=t3, in1=t4, op=A.subtract)         # oh
                S.activation(out=t1, in_=t1, func=AF.Relu)                      # relu(ow)
                V.scalar_tensor_tensor(out=t1, in0=t3, scalar=0.0, in1=t1,
                                       op0=A.max, op1=A.mult)                   # ovl
                # union reciprocal
                Gp.tensor_tensor(out=t2, in0=pev(0), in1=gtv(4), op=A.add)      # w1h1 + (w2h2+eps)
                V.tensor_tensor(out=t2, in0=t2, in1=t1, op=A.subtract)          # union
                V.reciprocal(out=t2, in_=t2)
                V.tensor_tensor(out=t1, in0=t1, in1=t2, op=A.mult)              # iou
                # enclosing box diag^2
                Gp.tensor_tensor(out=t2, in0=ppv(2), in1=gtv(2), op=A.max)
                V.tensor_tensor(out=t4, in0=ppv(0), in1=gtv(0), op=A.min)
                Gp.tensor_tensor(out=t2, in0=t2, in1=t4, op=A.subtract)         # ew
                V.tensor_tensor(out=t4, in0=ppv(3), in1=gtv(3), op=A.max)
                Gp.tensor_tensor(out=t5, in0=ppv(1), in1=gtv(1), op=A.min)
                V.tensor_tensor(out=t4, in0=t4, in1=t5, op=A.subtract)          # eh
                S.square(t2, t2)
                S.square(t4, t4)
                Gp.tensor_tensor(out=t2, in0=t2, in1=t4, op=A.add)              # ew2+eh2
                V.tensor_scalar(out=t2, in0=t2, scalar1=4.0, scalar2=4.0 * EPS,
                                op0=A.mult, op1=A.add)
                V.reciprocal(out=t2, in_=t2)                                    # 0.25/(enc+eps)
                # center distance
                Gp.tensor_tensor(out=t4, in0=gtv(5), in1=pev(1), op=A.subtract)
                V.tensor_tensor(out=t5, in0=gtv(6), in1=pev(2), op=A.subtract)
                S.square(t4, t4)
                S.square(t5, t5)
                Gp.tensor_tensor(out=t4, in0=t4, in1=t5, op=A.add)              # 4*rho2
                V.tensor_tensor(out=t2, in0=t4, in1=t2, op=A.mult)              # rho2/enc
                # aspect-ratio penalty
                Gp.tensor_tensor(out=t4, in0=gtv(7), in1=pev(3), op=A.subtract)
                S.activation(out=t4, in_=t4, func=AF.Square, scale=0.6366197723675814)  # wh
                V.tensor_tensor(out=t5, in0=t4, in1=t1, op=A.subtract)          # wh - iou
                V.tensor_scalar_add(t5, t5, 1.0 + EPS)
                V.reciprocal(out=t5, in_=t5)
                S.square(t6, t4)                                                # wh^2
                Gp.tensor_tensor(out=t5, in0=t6, in1=t5, op=A.mult)             # alpha*wh
                V.tensor_tensor(out=t1, in0=t1, in1=t2, op=A.subtract)
                Gp.tensor_tensor(out=t1, in0=t1, in1=t5, op=A.subtract)         # ciou
                V.tensor_scalar(out=ovs, in0=t1, scalar1=0.0, scalar2=1.0,
                                op0=A.max, op1=A.min)                           # overlaps
                # align = score * ov^6
                S.square(t2, ovs)
                S.square(t4, t2)
                Gp.tensor_tensor(out=t4, in0=t4, in1=t2, op=A.mult)             # ov^6
                V.tensor_tensor(out=als, in0=bs[:, c0:c1, :], in1=t4, op=A.mult)
                # in-gts mask (exact delta form) -- pad folded into bounds
                Gp.tensor_tensor(out=t2, in0=pxv(0), in1=gtv(8), op=A.subtract)     # px-x1t
                V.scalar_tensor_tensor(out=t4, in0=gtv(10), scalar=0.0, in1=pxv(0),
                                       op0=A.bypass, op1=A.subtract)                # x2t-px
                Gp.tensor_tensor(out=t2, in0=t2, in1=t4, op=A.min)
                V.tensor_tensor(out=t4, in0=pxv(1), in1=gtv(9), op=A.subtract)      # py-y1t
                Gp.scalar_tensor_tensor(out=t5, in0=gtv(11), scalar=0.0, in1=pxv(1),
                                        op0=A.bypass, op1=A.subtract)               # y2t-py
                V.tensor_tensor(out=t4, in0=t4, in1=t5, op=A.min)
                Gp.tensor_tensor(out=t2, in0=t2, in1=t4, op=A.min)
                V.tensor_scalar(out=igs, in0=t2, scalar1=1e-9, scalar2=None,
                                op0=A.is_gt)
                # metrics
                mtt = temps.tile(sh, F32, tag="mtt")
                V.tensor_tensor(out=mtt, in0=als, in1=igs, op=A.mult)

                # forward transpose of metrics into G-major rows
                for cs in range(0, NB, 4):
                    cw = min(4, NB - cs)
                    tps = psum.tile([G, 4, PP], F32, tag="fwdt")
                    for j in range(cw):
                        nc.tensor.transpose(tps[:, j, :], mtt[:, cs + j, :],
                                            ident[0:PP, 0:PP])
                    cc = c0 + cs
                    nc.scalar.copy(
                        mtg[:, cc * PP:(cc + cw) * PP],
                        tps[:, 0:cw, :].rearrange("p a b -> p (a b)"))

            if DEBUG:
                nc.sync.dma_start(out=_DBG["dmtg"][img], in_=mtg)
                for nm, tl in [("dov", ov), ("dbs", bs), ("dig", ig), ("dal", al)]:
                    nc.sync.dma_start(
                        out=_DBG[nm][img].rearrange("(c q) g -> q c g", q=PP), in_=tl)
            # ---------------- top-13 (exact jax.lax.top_k semantics) ----------------
            m1 = small.tile([G, 8], F32, tag="m1")
            m2 = small.tile([G, 8], F32, tag="m2")
            nc.vector.max(out=m1, in_=mtg)
            nc.vector.match_replace(out=mtg, in_to_replace=m1, in_values=mtg,
                                    imm_value=-1.0)
            nc.vector.max(out=m2, in_=mtg)
            nc.vector.memset(m2[:, 5:8], -1.0)
            nc.vector.match_replace(out=mtg, in_to_replace=m2, in_values=mtg,
                                    imm_value=-1.0)

            # ---------------- transpose marks back to P-major ----------------
            mark = perimg.tile([PP, NCH, G], F32, tag="bsmark")
            for cs in range(0, NCH, 5):
                tps = psum.tile([PP, 5, G], F32, tag="bwdt")
                for j in range(5):
                    c = cs + j
                    nc.tensor.transpose(tps[:, j, :], mtg[:, c * PP:(c + 1) * PP],
                                        ident[0:G, 0:G])
                nc.scalar.copy(mark[:, cs:cs + 5, :], tps)

            # ---------------- assignment resolution ----------------
            V, Gp = nc.vector, nc.vector
            pos = mark  # in-place: pos = (mark == -1) * ig
            V.scalar_tensor_tensor(out=pos, in0=mark, scalar=-1.0, in1=ig,
                                   op0=A.is_equal, op1=A.mult)
            fgi = small.tile([PP, NCH], F32, tag="fgi")
            V.tensor_reduce(out=fgi, in_=pos, axis=AX.X, op=A.add)
            multi = small.tile([PP, NCH], F32, tag="multi")
            V.tensor_scalar(out=multi, in0=fgi, scalar1=1.0, scalar2=None, op0=A.is_gt)

            cmx = small.tile([PP, NCH], F32, tag="cmx")
            V.tensor_reduce(out=cmx, in_=ov, axis=AX.X, op=A.max)
            eqm = scr[0:PP, :, 0:G]  # scr is dead after the BS gather
            Gp.tensor_tensor(out=eqm, in0=ov,
                             in1=cmx[:, :, None].to_broadcast([PP, NCH, G]),
                             op=A.is_equal)
            Gp.tensor_tensor(out=eqm, in0=eqm,
                             in1=gdesc[0:PP, None, :].to_broadcast([PP, NCH, G]),
                             op=A.mult)
            bmx = small.tile([PP, NCH], F32, tag="bmx")
            V.tensor_reduce(out=bmx, in_=eqm, axis=AX.X, op=A.max)
            bestg = small.tile([PP, NCH], F32, tag="bestg")
            V.tensor_scalar(out=bestg, in0=bmx, scalar1=-1.0, scalar2=float(G),
                            op0=A.mult, op1=A.add)
            ismax = eqm
            Gp.tensor_tensor(out=ismax, in0=giof[0:PP, None, :].to_broadcast([PP, NCH, G]),
                             in1=bestg[:, :, None].to_broadcast([PP, NCH, G]),
                             op=A.is_equal)
            notm = small.tile([PP, NCH], F32, tag="notm")
            V.tensor_scalar(out=notm, in0=multi, scalar1=-1.0, scalar2=1.0,
                            op0=A.mult, op1=A.add)
            Gp.tensor_tensor(out=ismax, in0=ismax,
                             in1=multi[:, :, None].to_broadcast([PP, NCH, G]), op=A.mult)
            V.tensor_tensor(out=pos, in0=pos,
                            in1=notm[:, :, None].to_broadcast([PP, NCH, G]), op=A.mult)
            V.tensor_tensor(out=pos, in0=pos, in1=ismax, op=A.add)
            fgf = small.tile([PP, NCH], F32, tag="fgf")
            V.tensor_reduce(out=fgf, in_=pos, axis=AX.X, op=A.add)
            fgb = small.tile([PP, NCH], U8, tag="fgb")
            V.tensor_scalar(out=fgb, in0=fgf, scalar1=0.0, scalar2=None, op0=A.is_gt)
            if DEBUG:
                nc.sync.dma_start(
                    out=_DBG["dpos"][img].rearrange("(c q) g -> q c g", q=PP),
                    in_=pos)



            # gather gt box + label via one-hot contraction (pos has <=1
            # nonzero per prior); background keeps gt[0] like the reference.
            gsc = ig  # ig is dead after pos was built
            bxl = small.tile([PP, NCH, 5], F32, tag="bxl")
            planes = [0, 1, 2, 3, 12]
            for j, pk in enumerate(planes):
                eng = V if j % 2 == 0 else Gp
                eng.tensor_tensor(out=gsc, in0=pos,
                                  in1=gt[:, pk:pk + 1, :].to_broadcast([PP, NCH, G]),
                                  op=A.mult)
                sel = bxl[:, :, j]
                V.tensor_reduce(out=sel, in_=gsc, axis=AX.X, op=A.add)
                fill = small.tile([PP, NCH], F32, tag="fill")
                nc.scalar.copy(fill, gt[:, pk, 0:1].to_broadcast([PP, NCH]))
                V.copy_predicated(out=fill, mask=fgb, data=sel)
                V.tensor_copy(sel, fill)

            # ---------------- normalizer ----------------
            am = al
            Gp.tensor_tensor(out=am, in0=al, in1=pos, op=A.mult)
            om = ov
            Gp.tensor_tensor(out=om, in0=ov, in1=pos, op=A.mult)
            pvv = small.tile([PP, 2 * G], F32, tag="pvv")
            V.tensor_reduce(out=pvv[:, 0:G], in_=am.rearrange("p c g -> p g c"),
                            axis=AX.X, op=A.max)
            V.tensor_reduce(out=pvv[:, G:2 * G], in_=om.rearrange("p c g -> p g c"),
                            axis=AX.X, op=A.max)
            tpv = psum1.tile([2 * G, PP], F32, tag="tpv")
            nc.tensor.transpose(tpv, pvv, ident[0:PP, 0:PP])
            spv = small.tile([2 * G, PP], F32, tag="spv")
            nc.scalar.copy(spv, tpv)
            col = small.tile([2 * G, 1], F32, tag="col")
            V.tensor_reduce(out=col, in_=spv, axis=AX.X, op=A.max)
            tcol = psum1.tile([1, 2 * G], F32, tag="dtp")
            nc.tensor.transpose(tcol, col, ident[0:2 * G, 0:2 * G])
            rowpa = small.tile([1, 2 * G], F32, tag="rowpa")
            nc.scalar.copy(rowpa, tcol)
            rr = small.tile([1, G], F32, tag="rr")
            V.tensor_scalar_add(rr, rowpa[:, 0:G], EPS)
            V.reciprocal(out=rr, in_=rr)
            V.tensor_tensor(out=rr, in0=rowpa[:, G:2 * G], in1=rr, op=A.mult)
            rbc = small.tile([PP, G], F32, tag="rbc")
            nc.sync.dma_start(out=rrsc[img, :], in_=rr)
            rbc_src = bass.AP(tensor=rrsc.ap().tensor, offset=img * G,
                              ap=[[0, PP], [1, G]])
            nc.sync.dma_start(out=rbc, in_=rbc_src)
            Gp.tensor_tensor(out=am, in0=am,
                             in1=rbc[:, None, :].to_broadcast([PP, NCH, G]), op=A.mult)
            norm = small.tile([PP, NCH], F32, tag="norm")
            V.tensor_reduce(out=norm, in_=am, axis=AX.X, op=A.max)

            # ---------------- outputs ----------------
            labi = outp.tile([PP, NCH], I32, tag="labi")
            nc.vector.tensor_copy(labi, bxl[:, :, 4])
            nc.sync.dma_start(out=olab[img].rearrange("(c q) -> q c", q=PP), in_=labi)
            nc.sync.dma_start(out=obox[img].rearrange("(c q) j -> q c j", q=PP),
                              in_=bxl[:, :, 0:4])
            nc.sync.dma_start(out=ofg[img].rearrange("(c q) -> q c", q=PP), in_=fgb)

            for blk in range(NBLK):
                c0 = blk * NB
                c1 = c0 + NB
                eng = nc.vector
                sco = outp.tile([PP, NB, C], F32, tag="sco")
                eng.tensor_tensor(out=sco,
                                  in0=clsf[0:PP, None, :].to_broadcast([PP, NB, C]),
                                  in1=bxl[:, c0:c1, 4][:, :, None].to_broadcast([PP, NB, C]),
                                  op=A.is_equal)
                eng2 = nc.vector
                eng2.tensor_tensor(out=sco, in0=sco,
                                   in1=norm[:, c0:c1, None].to_broadcast([PP, NB, C]),
                                   op=A.mult)
                nc.sync.dma_start(
                    out=osco[img].rearrange("(c q) k -> q c k", q=PP)[:, c0:c1, :],
                    in_=sco)


_NC_CACHE = None


def _get_nc():
    global _NC_CACHE
    if _NC_CACHE is None:
        nc = bass.Bass()
        _NC_CACHE = _emit(nc)
    return _NC_CACHE


def _make_cst():
    cst = np.zeros((128, 289), np.float32)
    cst[:, 0:128] = np.eye(128, dtype=np.float32)
    cst[:, 128:208] = np.arange(C, dtype=np.float32)[None, :]
    cst[:, 208:248] = np.arange(G, dtype=np.float32)[None, :]
    cst[:, 248:288] = float(G) - np.arange(G, dtype=np.float32)[None, :]
    cst[:, 288] = np.arange(128, dtype=np.float32)
    return cst


def kernel(pred_bboxes, pred_scores, priors, gt_labels, gt_bboxes, pad_bbox_flag):
    lab_dt = gt_labels.dtype
    nc = _get_nc()
    in_maps = []
    for k in range(NCORE):
        s = slice(k * IPC, (k + 1) * IPC)
        in_maps.append({
            "pb": np.ascontiguousarray(pred_bboxes[s], np.float32),
            "ps": np.ascontiguousarray(pred_scores[s], np.float32),
            "pri": np.ascontiguousarray(priors, np.float32),
            "gl": np.ascontiguousarray(gt_labels[s, :, 0].astype(np.int32)),
            "gb": np.ascontiguousarray(gt_bboxes[s], np.float32),
            "pad": np.ascontiguousarray(pad_bbox_flag[s, :, 0], np.float32),
            "cst": _make_cst(),
        })
    res = run_bass_kernel_spmd(nc, in_maps, core_ids=list(range(NCORE)),
                               trace=TRACE, **(RUN_KWARGS or {}))
    global LAST_RESULT
    LAST_RESULT = res
    outs = res.results
    labels = np.concatenate([o["olab"] for o in outs], 0)
    bboxes = np.concatenate([o["obox"] for o in outs], 0)
    scores = np.concatenate([o["osco"] for o in outs], 0)
    fg = np.concatenate([o["ofg"] for o in outs], 0).astype(bool)
    if lab_dt != labels.dtype and np.issubdtype(lab_dt, np.integer):
        labels = labels.astype(np.int32)
    return labels, bboxes, scores, fg
